# revision 1
# baseline (speedup 1.0000x reference)
"""BitLinear kernel for Trainium2 (8 NeuronCores, tensor-parallel).

Computes: out = x @ (sign(w) * mean(|w|, axis=1, keepdims=True)).T
  x      : [4, 2048, 4096] f32
  weight : [4096, 4096] f32
  out    : [4, 2048, 4096] f32

Strategy (per sharding hint): shard weight rows (out features) 8-way.
Each core:
  - receives the full activations as x.T in bf16, pre-tiled on host so
    every DMA is a contiguous 256KB chunk with 2KB-per-partition packets
    (xTp [8 pairs, 32 k-tiles, 128, 1024]), plus its weight shard in
    bf16 in natural layout (wn, for the per-row abs-mean scales) and
    k-tile-paired transposed layout (wTt, for the sign operand). bf16
    weights are sign-safe, and the scale's mean over 4096 |w| values
    averages bf16 rounding noise down to ~3e-5 relative.
  - binarizes on device: S = sign(w) in bf16 (exact +-1 / 0); s =
    mean|w| accumulated in f32 by the DVE reduction,
  - matmuls with S as the 128x128 stationary operand and x.T as the
    moving operand ([128, 512] moving tiles), accumulating over k in
    PSUM (f32); the f32 per-feature scale is applied by the scalar
    engine while evicting PSUM -> SBUF; stores write the feature-major
    shard outT [512, 8192].
Host gathers the 8 outT shards -> [4096, 8192] -> transpose -> out.

The matmul stream runs at the PE bf16 fill-rate floor (~216ns per
128x512 matmul). x loads + scale-weight loads are chained in emission
order on the sync engine's DMA queue (startup is HBM-bound and the
order matters); sign-weight loads are interleaved per k-pair; output
stores ride the scalar engine's queue so eviction waits never block x
loads. The first block pair is computed j-outer across all 8 PSUM banks
so the PE keeps pace with the HBM-limited startup stream.
"""

import os
from contextlib import ExitStack

import numpy as np
import ml_dtypes

import concourse.bass as bass
import concourse.mybir as mybir
import concourse.tile as tile
from concourse import bacc, bass_utils

P = 128                 # SBUF partitions / PE array dim
D_IN = 4096             # contraction dim (in features)
D_OUT = 4096            # out features
M_TOT = 8192            # tokens (4*2048)
N_CORES = 8
N_SHARD = D_OUT // N_CORES      # 512 out features per core
K_TILES = D_IN // P             # 32
M_BLK = 512                     # moving free dim per matmul
M_BLKS = M_TOT // M_BLK         # 16
M_PAIRS = M_BLKS // 2           # 8 (x is loaded in block pairs)
N_TILES = N_SHARD // P          # 4

_CACHE = {}
LAST_RESULTS = None  # BassKernelResults of the most recent run (for test harness)


def _install_ntff_hook():
    """Register the ctypes NTFF profiling hook under antenv.axon_hooks so
    run_bass_kernel_spmd(trace=True) can capture device profiles under axon.
    No-op if already present or the .so lacks the symbols."""
    import contextlib
    import ctypes
    import sys
    import types

    try:
        from antenv.axon_hooks import get_axon_ntff_profile_hook  # noqa: F401

        return True
    except ImportError:
        pass

    so_path = "/opt/axon/libaxon_pjrt.so"
    if not os.path.exists(so_path):
        return False
    lib = ctypes.CDLL(so_path)
    if not hasattr(lib, "axon_start_nrt_profile"):
        return False
    lib.axon_start_nrt_profile.argtypes = [
        ctypes.POINTER(ctypes.c_int64),
        ctypes.c_size_t,
    ]
    lib.axon_start_nrt_profile.restype = ctypes.c_int64
    lib.axon_stop_nrt_profile.argtypes = [ctypes.c_char_p]
    lib.axon_stop_nrt_profile.restype = ctypes.c_int64

    @contextlib.contextmanager
    def _hook(output_dir, device_ids):
        import jax

        jax.devices()
        if device_ids:
            ids = (ctypes.c_int64 * len(device_ids))(*device_ids)
            rc = lib.axon_start_nrt_profile(ids, len(device_ids))
        else:
            rc = lib.axon_start_nrt_profile(None, 0)
        if rc != 0:
            raise RuntimeError(f"axon_start_nrt_profile rc={rc}")
        try:
            yield
        finally:
            n = lib.axon_stop_nrt_profile(str(output_dir).encode())
            print(f"ntff profile: {n} file(s) written to {output_dir}")

    mod = types.ModuleType("antenv.axon_hooks")
    _state = {"hook": _hook}
    mod.set_axon_ntff_profile_hook = lambda h: _state.__setitem__("hook", h)
    mod.get_axon_ntff_profile_hook = lambda: _state["hook"]
    sys.modules["antenv.axon_hooks"] = mod
    import antenv

    antenv.axon_hooks = mod

    # artifact upload reaches for a cloud bucket that isn't available here
    bass_utils.upload_artifacts = lambda tmpdir: f"local:{tmpdir}"
    return True


def _build_nc():
    nc = bacc.Bacc(
        "TRN2", target_bir_lowering=False, debug=False, num_devices=N_CORES,
        enable_partition_id=False,
    )
    # x pre-tiled on host: xTp[q, j, p, m] = x.T[j*128+p, q*1024+m], so each
    # (q, j) DMA is a fully contiguous 256KB read with 2KB-per-partition
    # packets (1KB packets run the DMA queue ~40% slower).
    xTp = nc.dram_tensor(
        "xTp", [M_PAIRS, K_TILES, P, 2 * M_BLK], mybir.dt.bfloat16,
        kind="ExternalInput",
    )
    wn = nc.dram_tensor("wn", [N_SHARD, D_IN], mybir.dt.bfloat16, kind="ExternalInput")
    # w.T pre-tiled in k-tile pairs: wTt[jj, p, h*512+n] = w.T[(2*jj+h)*128+p, n]
    # so each DMA has 2KB-per-partition packets.
    wTt = nc.dram_tensor(
        "wTt", [K_TILES // 2, P, 2 * N_SHARD], mybir.dt.bfloat16,
        kind="ExternalInput",
    )
    outT = nc.dram_tensor(
        "outT", [N_SHARD, M_TOT], mybir.dt.float32, kind="ExternalOutput"
    )

    with tile.TileContext(nc) as tc, ExitStack() as ctx:
        spool = ctx.enter_context(tc.tile_pool(name="scales", bufs=1))
        wpool = ctx.enter_context(tc.tile_pool(name="wnat", bufs=2))
        wtpool = ctx.enter_context(tc.tile_pool(name="wtrans", bufs=6))
        sgpool = ctx.enter_context(tc.tile_pool(name="sign", bufs=1))
        xpool = ctx.enter_context(tc.tile_pool(name="xpair", bufs=2))
        opool = ctx.enter_context(tc.tile_pool(name="oblk", bufs=6))
        ppool = ctx.enter_context(tc.tile_pool(name="psum", bufs=8, space="PSUM"))

        # Queue assignment: sync = x loads + scale-weight loads (chained in
        # emission order so the FIFO queue is deterministic); scalar =
        # sign-weight loads, then output stores (which must wait on evictions
        # and would stall x loads).
        PAIR_W = 2 * M_BLK
        prev_sync_dma = [None]

        def sync_load(dst, src):
            dma = nc.sync.dma_start(dst, src)
            if prev_sync_dma[0] is not None:
                # add_dep_helper(waiter, dependency): this load is ordered
                # after the previous one on the sync queue.
                tile.add_dep_helper(
                    dma.ins, prev_sync_dma[0].ins, sync=False,
                    reason="sync DMA queue emission order",
                )
            prev_sync_dma[0] = dma
            return dma

        def issue_x_pair(q):
            xt = xpool.tile([P, K_TILES * PAIR_W], mybir.dt.bfloat16, tag="xpair")
            for j in range(K_TILES):
                sync_load(xt[:, j * PAIR_W : (j + 1) * PAIR_W], xTp[q, j, :, :])
            return xt

        def mm_block(pss, xt, b, ni, j):
            nc.tensor.matmul(
                pss[ni][:],
                S_all[:, j * N_SHARD + ni * P : j * N_SHARD + (ni + 1) * P],
                xt[:, j * PAIR_W + b * M_BLK : j * PAIR_W + b * M_BLK + M_BLK],
                start=(j == 0),
                stop=(j == K_TILES - 1),
            )

        def evict_block(pss, mb):
            # Evictions alternate between the scalar and vector engines so
            # the per-block eviction chain (and the kernel tail) is half as
            # long. Stores ride the scalar queue; for the final block the
            # sync queue (drained of x loads by then) takes half the store
            # triggers so the tail isn't serialized on one engine.
            last = mb == M_BLKS - 1
            for ni in range(N_TILES):
                ot = opool.tile([P, M_BLK], mybir.dt.float32, tag="ot", name="ot")
                dst = outT[ni * P : (ni + 1) * P, mb * M_BLK : (mb + 1) * M_BLK]
                if ni % 2 == 0:
                    nc.scalar.mul(ot[:], pss[ni][:], s_all[:, ni : ni + 1])
                else:
                    nc.vector.tensor_scalar_mul(
                        ot[:], pss[ni][:], s_all[:, ni : ni + 1]
                    )
                if last and ni % 2 == 1:
                    nc.sync.dma_start(dst, ot[:])
                else:
                    nc.scalar.dma_start(dst, ot[:])

        # Prologue: interleave sign-weight loads with the first x pair's
        # loads on the chained sync queue so the earliest matmuls are fed in
        # lockstep with minimal latency.
        S_all = sgpool.tile([P, K_TILES * N_SHARD], mybir.dt.bfloat16)
        xt0 = xpool.tile([P, K_TILES * PAIR_W], mybir.dt.bfloat16, tag="xpair")
        # Zero bias for the Sign activations as a plain SBUF tile (a float
        # bias would pull in a const-AP DRAM load during the preamble), and a
        # dummy 1-column sign to hoist the ACT LUT table load off the
        # critical path of the first real sign.
        zbias = spool.tile([P, 1], mybir.dt.float32)
        nc.gpsimd.memset(zbias[:], 0.0)
        nc.scalar.activation(
            S_all[:, 0:1], zbias[:], mybir.ActivationFunctionType.Sign,
            bias=zbias[:],
        )
        for jj in range(K_TILES // 2):
            wt_t = wtpool.tile([P, 2 * N_SHARD], mybir.dt.bfloat16)
            if jj == 0:
                # Split the first load/sign so the very first matmul (k-tile
                # 0, n-tile 0) is unblocked by a 32KB load + 128-col sign
                # instead of the full 512KB/1024-col pair, and slot the
                # first moving tile's load between the two sign chunks so
                # matmul #0's operands land back to back.
                sync_load(wt_t[:, 0:P], wTt[0, :, 0:P])
                nc.scalar.activation(
                    S_all[:, 0:P], wt_t[:, 0:P],
                    mybir.ActivationFunctionType.Sign, bias=zbias[:],
                )
                sync_load(xt0[:, 0:M_BLK], xTp[0, 0, :, 0:M_BLK])
                sync_load(wt_t[:, P:], wTt[0, :, P:])
                nc.scalar.activation(
                    S_all[:, P : 2 * N_SHARD], wt_t[:, P:],
                    mybir.ActivationFunctionType.Sign, bias=zbias[:],
                )
                sync_load(xt0[:, M_BLK:PAIR_W], xTp[0, 0, :, M_BLK:])
            else:
                sync_load(wt_t[:], wTt[jj, :, :])
                nc.scalar.activation(
                    S_all[:, 2 * jj * N_SHARD : (2 * jj + 2) * N_SHARD],
                    wt_t[:],
                    mybir.ActivationFunctionType.Sign,
                    bias=zbias[:],
                )
            for j in ((1,) if jj == 0 else (2 * jj, 2 * jj + 1)):
                sync_load(xt0[:, j * PAIR_W : (j + 1) * PAIR_W], xTp[0, j, :, :])

        # Per-out-feature scales: s[n] = mean_k |w[n, k]|, kept per n-tile as
        # a [128, 1] per-partition column (column i = n-tile i). Only needed
        # by the PSUM evictions (first one ~2 blocks in).
        # These ride the sync queue BEHIND the first x pair's loads so they
        # don't steal HBM bandwidth during the HBM-paced startup; they still
        # land well before the first eviction needs them.
        s_all = spool.tile([P, N_TILES], mybir.dt.float32)
        for i in range(N_TILES):
            wtile = wpool.tile([P, D_IN], mybir.dt.bfloat16)
            sync_load(wtile[:], wn[i * P : (i + 1) * P, :])
            nc.vector.reduce_sum(
                s_all[:, i : i + 1],
                wtile[:],
                axis=mybir.AxisListType.X,
                apply_absolute_value=True,
            )
        nc.vector.tensor_scalar_mul(s_all[:], s_all[:], 1.0 / D_IN)

        # Main loop: out.T[n, m] = sum_k S[k, n] * xT[k, m], scaled by s[n].
        # Pair 0 is computed j-outer across BOTH blocks (8 PSUM banks) so the
        # PE keeps pace with the HBM-limited startup stream; later pairs run
        # block-at-a-time j-outer (4 banks ping-ponging with the previous
        # block's draining 4).
        for q in range(M_PAIRS):
            xt = xt0 if q == 0 else issue_x_pair(q)
            if q == 0:
                pss2 = [
                    [
                        ppool.tile(
                            [P, M_BLK], mybir.dt.float32, tag="ps",
                            name=f"ps_{b}_{ni}",
                        )
                        for ni in range(N_TILES)
                    ]
                    for b in range(2)
                ]
                for j in range(K_TILES):
                    for b in range(2):
                        for ni in range(N_TILES):
                            mm_block(pss2[b], xt, b, ni, j)
                for b in range(2):
                    evict_block(pss2[b], b)
            else:
                for b in range(2):
                    last_blk = q == M_PAIRS - 1 and b == 1
                    pss = [
                        ppool.tile(
                            [P, M_BLK], mybir.dt.float32, tag="ps", name=f"ps{ni}"
                        )
                        for ni in range(N_TILES)
                    ]
                    if last_blk:
                        # ni-outer for the final block: each n-tile's stop
                        # matmul lands early, so its eviction + store overlap
                        # the remaining matmuls instead of serializing after
                        # the last one.
                        for ni in range(N_TILES):
                            for j in range(K_TILES):
                                mm_block(pss, xt, b, ni, j)
                    else:
                        for j in range(K_TILES):
                            for ni in range(N_TILES):
                                mm_block(pss, xt, b, ni, j)
                    evict_block(pss, 2 * q + b)

    nc.compile()
    return nc


def kernel(x, weight):
    global LAST_RESULTS
    nc = _CACHE.get("nc")
    if nc is None:
        nc = _CACHE["nc"] = _build_nc()

    x = np.asarray(x)
    weight = np.asarray(weight)
    orig_shape = x.shape

    # Host-side sharding/layout: xT in bf16 (replicated, pre-tiled so each
    # (pair, k-tile) chunk is contiguous), weight shard in both layouts.
    xT = x.reshape(M_TOT, D_IN).T  # [D_IN, M_TOT] view
    xTp = np.ascontiguousarray(
        xT.reshape(K_TILES, P, M_PAIRS, 2 * M_BLK)
        .transpose(2, 0, 1, 3)
        .astype(ml_dtypes.bfloat16)
    )  # [M_PAIRS, K_TILES, P, 1024]
    wt_full = np.ascontiguousarray(weight.T)  # [D_IN, D_OUT] f32
    in_maps = []
    for c in range(N_CORES):
        in_maps.append(
            {
                "xTp": xTp,
                "wn": np.ascontiguousarray(
                    weight[c * N_SHARD : (c + 1) * N_SHARD, :].astype(
                        ml_dtypes.bfloat16
                    )
                ),
                "wTt": np.ascontiguousarray(
                    wt_full[:, c * N_SHARD : (c + 1) * N_SHARD]
                    .reshape(K_TILES // 2, 2, P, N_SHARD)
                    .transpose(0, 2, 1, 3)
                    .reshape(K_TILES // 2, P, 2 * N_SHARD)
                    .astype(ml_dtypes.bfloat16)
                ),
            }
        )

    trace = bool(int(os.environ.get("BITLIN_TRACE", "0")))
    if trace:
        trace = _install_ntff_hook()
        base = os.environ.get("BITLIN_TRACE_DIR") or None
        if base:
            import tempfile

            os.makedirs(base, exist_ok=True)
            tmpdir = tempfile.mkdtemp(dir=base)
        else:
            tmpdir = None
    else:
        tmpdir = None
    res = bass_utils.run_bass_kernel_spmd(
        nc, in_maps, core_ids=list(range(N_CORES)), trace=trace, tmpdir=tmpdir
    )
    LAST_RESULTS = res

    outT_full = np.concatenate(
        [np.asarray(res.results[c]["outT"]) for c in range(N_CORES)], axis=0
    )  # [D_OUT, M_TOT] f32
    out = np.ascontiguousarray(outT_full.T).reshape(orig_shape).astype(np.float32)
    return out



# revision 2
# speedup vs baseline: 1.2932x; 1.2932x over previous
"""BitLinear kernel for Trainium2 (8 NeuronCores, tensor-parallel).

Computes: out = x @ (sign(w) * mean(|w|, axis=1, keepdims=True)).T
  x      : [4, 2048, 4096] f32
  weight : [4096, 4096] f32
  out    : [4, 2048, 4096] f32

Strategy: shard weight rows (out features) 8-way; each core computes a
[512, 8192] feature-major output shard.

Mixed-precision contraction (PE roofline is the bottleneck): the first
18 k-tiles run as bf16 matmuls (216ns per 128x512), the last 14 k-tiles
run as 7 fp8e4 DoubleRow pair-matmuls (2 k-tiles per instruction at
~241ns — 2x the contraction per instruction at +13% per-instruction
cost). sign(w) is exactly representable in both bf16 and fp8e4, so all
quantization error comes from the e4m3 x tiles: measured end-to-end
rel-err 1.76e-2 (gate 2e-2). Host computes signs and f32 scales; the
device applies the per-feature scale while evicting PSUM and stores the
output shard in bf16 (negligible extra error, half the store traffic).

Layouts (host-pretiled so every DMA is a contiguous 256KB chunk with
2KB-per-partition packets):
  xTp [8 pairs, 18, 128, 1024] bf16 — x.T k-tile chunks per 1024-token
      pair (two 512 blocks).
  xF  [8 pairs, 7, 128, 2048] fp8e4 — DoubleRow pairs: per partition
      [ktA blk0 512 | ktA blk1 512 | ktB blk0 | ktB blk1]; the matmul
      rhs AP is [128, 2 slots, 512] with slot stride 1024.
  sgB [128, 9216] bf16, sgF [128, 7168] fp8e4 — sign(w).T tiles,
      n-shard-major per k-tile; DR lhsT AP is [128, 2, 128], slot
      stride 512.
  sc  [128, 4] f32 — per-feature scales as n-tile columns.
  outT[512, 8192] bf16 — feature-major shard.

The matmul stream (16 blocks x 4 n-tiles x (18 bf16 + 7 DR)) is
~357us of PE time. x/sign loads chain in emission order on the sync
DMA queue (startup is HBM-paced); output stores ride the scalar
queue. Pair 0 is computed j-outer across both blocks (8 PSUM banks)
so the PE keeps pace with the HBM-limited startup stream.
"""

import os
from contextlib import ExitStack

import numpy as np
import ml_dtypes

import concourse.bass as bass
import concourse.mybir as mybir
import concourse.tile as tile
from concourse import bacc, bass_utils

P = 128                 # SBUF partitions / PE array dim
D_IN = 4096             # contraction dim (in features)
D_OUT = 4096            # out features
M_TOT = 8192            # tokens (4*2048)
N_CORES = 8
N_SHARD = D_OUT // N_CORES      # 512 out features per core
K_TILES = D_IN // P             # 32
NB = 18                         # bf16 k-tiles (0..17)
NFP = (K_TILES - NB) // 2       # 7 fp8 DoubleRow k-tile pairs (18..31)
M_BLK = 512                     # moving free dim per matmul
M_BLKS = M_TOT // M_BLK         # 16
M_PAIRS = M_BLKS // 2           # 8 (x is loaded in block pairs)
N_TILES = N_SHARD // P          # 4
PAIR_W = 2 * M_BLK              # 1024

_CACHE = {}
LAST_RESULTS = None  # BassKernelResults of the most recent run (for test harness)


def _install_ntff_hook():
    """Register the ctypes NTFF profiling hook under antenv.axon_hooks so
    run_bass_kernel_spmd(trace=True) can capture device profiles under axon.
    No-op if already present or the .so lacks the symbols."""
    import contextlib
    import ctypes
    import sys
    import types

    try:
        from antenv.axon_hooks import get_axon_ntff_profile_hook  # noqa: F401

        return True
    except ImportError:
        pass

    so_path = "/opt/axon/libaxon_pjrt.so"
    if not os.path.exists(so_path):
        return False
    lib = ctypes.CDLL(so_path)
    if not hasattr(lib, "axon_start_nrt_profile"):
        return False
    lib.axon_start_nrt_profile.argtypes = [
        ctypes.POINTER(ctypes.c_int64),
        ctypes.c_size_t,
    ]
    lib.axon_start_nrt_profile.restype = ctypes.c_int64
    lib.axon_stop_nrt_profile.argtypes = [ctypes.c_char_p]
    lib.axon_stop_nrt_profile.restype = ctypes.c_int64

    @contextlib.contextmanager
    def _hook(output_dir, device_ids):
        import jax

        jax.devices()
        if device_ids:
            ids = (ctypes.c_int64 * len(device_ids))(*device_ids)
            rc = lib.axon_start_nrt_profile(ids, len(device_ids))
        else:
            rc = lib.axon_start_nrt_profile(None, 0)
        if rc != 0:
            raise RuntimeError(f"axon_start_nrt_profile rc={rc}")
        try:
            yield
        finally:
            n = lib.axon_stop_nrt_profile(str(output_dir).encode())
            print(f"ntff profile: {n} file(s) written to {output_dir}")

    mod = types.ModuleType("antenv.axon_hooks")
    _state = {"hook": _hook}
    mod.set_axon_ntff_profile_hook = lambda h: _state.__setitem__("hook", h)
    mod.get_axon_ntff_profile_hook = lambda: _state["hook"]
    sys.modules["antenv.axon_hooks"] = mod
    import antenv

    antenv.axon_hooks = mod

    # artifact upload reaches for a cloud bucket that isn't available here
    bass_utils.upload_artifacts = lambda tmpdir: f"local:{tmpdir}"
    return True


def _build_nc():
    nc = bacc.Bacc(
        "TRN2", target_bir_lowering=False, debug=False, num_devices=N_CORES,
        enable_partition_id=False,
    )
    xTp = nc.dram_tensor(
        "xTp", [M_PAIRS, NB, P, PAIR_W], mybir.dt.bfloat16, kind="ExternalInput"
    )
    xF = nc.dram_tensor(
        "xF", [M_PAIRS, NFP, P, 2 * PAIR_W], mybir.dt.float8e4,
        kind="ExternalInput",
    )
    sgB = nc.dram_tensor(
        "sgB", [P, NB * N_SHARD], mybir.dt.bfloat16, kind="ExternalInput"
    )
    sgF = nc.dram_tensor(
        "sgF", [P, NFP * 2 * N_SHARD], mybir.dt.float8e4, kind="ExternalInput"
    )
    sc = nc.dram_tensor("sc", [P, N_TILES], mybir.dt.float32, kind="ExternalInput")
    outT = nc.dram_tensor(
        "outT", [N_SHARD, M_TOT], mybir.dt.bfloat16, kind="ExternalOutput"
    )

    with tile.TileContext(nc) as tc, ExitStack() as ctx:
        spool = ctx.enter_context(tc.tile_pool(name="scales", bufs=1))
        sgpool = ctx.enter_context(tc.tile_pool(name="sign", bufs=1))
        xpoolB = ctx.enter_context(tc.tile_pool(name="xb", bufs=2))
        xpoolF = ctx.enter_context(tc.tile_pool(name="xf", bufs=2))
        opool = ctx.enter_context(tc.tile_pool(name="oblk", bufs=8))
        ppool = ctx.enter_context(tc.tile_pool(name="psum", bufs=8, space="PSUM"))

        # sync queue = all input loads, chained in emission order so the
        # FIFO order is deterministic and startup is HBM-paced in the
        # order the PE consumes tiles. scalar queue = output stores.
        prev_sync_dma = [None]

        def sync_load(dst, src):
            dma = nc.sync.dma_start(dst, src)
            if prev_sync_dma[0] is not None:
                tile.add_dep_helper(
                    dma.ins, prev_sync_dma[0].ins, sync=False,
                    reason="sync DMA queue emission order",
                )
            prev_sync_dma[0] = dma
            return dma

        # Persistent sign tiles + scales
        sgB_t = sgpool.tile([P, NB, N_SHARD], mybir.dt.bfloat16)
        sgF_t = sgpool.tile([P, NFP, 2, N_SHARD], mybir.dt.float8e4)
        sct = spool.tile([P, N_TILES], mybir.dt.float32)

        def issue_x_pair(q, xb=None, xf=None):
            if xb is None:
                xb = xpoolB.tile([P, NB, PAIR_W], mybir.dt.bfloat16, tag="xb")
            if xf is None:
                xf = xpoolF.tile([P, NFP, 2, PAIR_W], mybir.dt.float8e4, tag="xf")
            for j in range(NB):
                sync_load(xb[:, j, :], xTp[q, j, :, :])
            for jj in range(NFP):
                sync_load(xf[:, jj, :, :], xF[q, jj, :, :])
            return xb, xf

        def mm_b(pss, xb, b, ni, j):
            nc.tensor.matmul(
                pss[ni][:],
                sgB_t[:, j, ni * P : (ni + 1) * P],
                xb[:, j, b * M_BLK : (b + 1) * M_BLK],
                start=(j == 0),
                stop=False,
            )

        def mm_f(pss, xf, b, ni, jj):
            nc.tensor.matmul(
                pss[ni][:],
                sgF_t[:, jj, :, ni * P : (ni + 1) * P],
                xf[:, jj, :, b * M_BLK : (b + 1) * M_BLK],
                start=False,
                stop=(jj == NFP - 1),
                perf_mode=mybir.MatmulPerfMode.DoubleRow,
            )

        def evict_block(pss, opair, b, last=False):
            # Evictions alternate between the scalar and vector engines so
            # the per-block eviction chain (and the kernel tail) is half as
            # long. Stores (issued on b==1 once the [128,1024] pair tile is
            # complete) ride the scalar queue; the final pair splits store
            # triggers across the sync queue (drained of x loads by then).
            for ni in range(N_TILES):
                dst = opair[ni][:, b * M_BLK : (b + 1) * M_BLK]
                if ni % 2 == 0:
                    nc.scalar.mul(dst, pss[ni][:], sct[:, ni : ni + 1])
                else:
                    nc.vector.tensor_scalar_mul(dst, pss[ni][:], sct[:, ni : ni + 1])

        def store_pair(q, opair, last=False):
            for ni in range(N_TILES):
                dst = outT[ni * P : (ni + 1) * P, q * PAIR_W : (q + 1) * PAIR_W]
                if last and ni % 2 == 1:
                    nc.sync.dma_start(dst, opair[ni][:])
                else:
                    nc.scalar.dma_start(dst, opair[ni][:])

        # ---- Prologue: chain sign chunks just ahead of the x tiles that
        # consume them so the earliest matmuls are fed with minimal latency.
        xb0 = xpoolB.tile([P, NB, PAIR_W], mybir.dt.bfloat16, tag="xb")
        xf0 = xpoolF.tile([P, NFP, 2, PAIR_W], mybir.dt.float8e4, tag="xf")
        # k-tile 0 signs, then its x pair split in halves so matmul #0 is
        # unblocked by minimal bytes.
        sync_load(sgB_t[:, 0, :], sgB[:, 0:N_SHARD])
        sync_load(xb0[:, 0, 0:M_BLK], xTp[0, 0, :, 0:M_BLK])
        sync_load(xb0[:, 0, M_BLK:PAIR_W], xTp[0, 0, :, M_BLK:])
        # remaining bf16 signs in 2-k-tile chunks, interleaved with pair-0 x
        j = 1
        while j < NB:
            j2 = min(j + 2, NB)
            sync_load(sgB_t[:, j:j2, :], sgB[:, j * N_SHARD : j2 * N_SHARD])
            for jx in range(j, j2):
                sync_load(xb0[:, jx, :], xTp[0, jx, :, :])
            j = j2
        # fp8 signs in 2-pair chunks interleaved with pair-0 fp8 x
        jj = 0
        while jj < NFP:
            jj2 = min(jj + 2, NFP)
            sync_load(
                sgF_t[:, jj:jj2, :, :],
                sgF[:, jj * 2 * N_SHARD : jj2 * 2 * N_SHARD],
            )
            for jx in range(jj, jj2):
                sync_load(xf0[:, jx, :, :], xF[0, jx, :, :])
            jj = jj2
        # scales ride behind pair-0 (only needed at first eviction)
        sync_load(sct[:], sc[:, :])

        # ---- Main loop
        for q in range(M_PAIRS):
            xb, xf = (xb0, xf0) if q == 0 else issue_x_pair(q)
            opair = [
                opool.tile([P, PAIR_W], mybir.dt.bfloat16, tag="op", name=f"op{ni}")
                for ni in range(N_TILES)
            ]
            if q == 0:
                # j-outer across BOTH blocks (8 PSUM banks) so the PE keeps
                # pace with the HBM-limited startup stream.
                pss2 = [
                    [
                        ppool.tile(
                            [P, M_BLK], mybir.dt.float32, tag="ps",
                            name=f"ps_{b}_{ni}",
                        )
                        for ni in range(N_TILES)
                    ]
                    for b in range(2)
                ]
                for j in range(NB):
                    for b in range(2):
                        for ni in range(N_TILES):
                            mm_b(pss2[b], xb, b, ni, j)
                for jj in range(NFP):
                    for b in range(2):
                        for ni in range(N_TILES):
                            mm_f(pss2[b], xf, b, ni, jj)
                for b in range(2):
                    evict_block(pss2[b], opair, b)
                store_pair(q, opair)
            else:
                for b in range(2):
                    last_blk = q == M_PAIRS - 1 and b == 1
                    pss = [
                        ppool.tile(
                            [P, M_BLK], mybir.dt.float32, tag="ps", name=f"ps{ni}"
                        )
                        for ni in range(N_TILES)
                    ]
                    if last_blk:
                        # ni-outer for the final block: each n-tile's stop
                        # matmul lands early, so its eviction + store overlap
                        # the remaining matmuls instead of serializing after
                        # the last one.
                        for ni in range(N_TILES):
                            for j in range(NB):
                                mm_b(pss, xb, b, ni, j)
                            for jj in range(NFP):
                                mm_f(pss, xf, b, ni, jj)
                    else:
                        for j in range(NB):
                            for ni in range(N_TILES):
                                mm_b(pss, xb, b, ni, j)
                        for jj in range(NFP):
                            for ni in range(N_TILES):
                                mm_f(pss, xf, b, ni, jj)
                    evict_block(pss, opair, b, last=last_blk)
                store_pair(q, opair, last=(q == M_PAIRS - 1))

    nc.compile()
    return nc


def kernel(x, weight):
    global LAST_RESULTS
    nc = _CACHE.get("nc")
    if nc is None:
        nc = _CACHE["nc"] = _build_nc()

    x = np.asarray(x)
    weight = np.asarray(weight)
    orig_shape = x.shape

    KB = NB * P  # 2304 contraction cols in bf16

    # Host-side layout: x.T pre-tiled; bf16 for k-tiles 0..NB-1, e4m3 for
    # the DoubleRow k-tile pairs. Each (pair, chunk) DMA source is a fully
    # contiguous 256KB read with 2KB-per-partition packets.
    xT = x.reshape(M_TOT, D_IN).T  # [D_IN, M_TOT] view
    xTp = np.ascontiguousarray(
        xT[:KB].reshape(NB, P, M_PAIRS, PAIR_W)
        .transpose(2, 0, 1, 3)
        .astype(ml_dtypes.bfloat16)
    )  # [M_PAIRS, NB, P, 1024]
    # [NFP, 2, P, M_PAIRS, 1024] -> [M_PAIRS, NFP, P, 2, 1024] -> flat 2048
    xF = np.ascontiguousarray(
        xT[KB:].reshape(NFP, 2, P, M_PAIRS, PAIR_W)
        .transpose(3, 0, 2, 1, 4)
        .reshape(M_PAIRS, NFP, P, 2 * PAIR_W)
        .astype(ml_dtypes.float8_e4m3fn)
    )

    SgT = np.sign(weight.T)  # [D_IN, D_OUT] f32, sign exact
    s_full = np.abs(weight.astype(np.float64)).mean(axis=1).astype(np.float32)

    in_maps = []
    for c in range(N_CORES):
        n0 = c * N_SHARD
        shard = SgT[:, n0 : n0 + N_SHARD]  # [D_IN, 512]
        # sgB[p, j*512+n] = sign(wT[j*128+p, n0+n])
        sgB = np.ascontiguousarray(
            shard[:KB].reshape(NB, P, N_SHARD)
            .transpose(1, 0, 2)
            .reshape(P, NB * N_SHARD)
            .astype(ml_dtypes.bfloat16)
        )
        # sgF[p, jj*1024 + t*512 + n] = sign(wT[(NB+2jj+t)*128+p, n0+n])
        sgF = np.ascontiguousarray(
            shard[KB:].reshape(NFP, 2, P, N_SHARD)
            .transpose(2, 0, 1, 3)
            .reshape(P, NFP * 2 * N_SHARD)
            .astype(ml_dtypes.float8_e4m3fn)
        )
        scl = np.ascontiguousarray(
            s_full[n0 : n0 + N_SHARD].reshape(N_TILES, P).T
        )  # [128, 4] f32
        in_maps.append({"xTp": xTp, "xF": xF, "sgB": sgB, "sgF": sgF, "sc": scl})

    trace = bool(int(os.environ.get("BITLIN_TRACE", "0")))
    if trace:
        trace = _install_ntff_hook()
        base = os.environ.get("BITLIN_TRACE_DIR") or None
        if base:
            import tempfile

            os.makedirs(base, exist_ok=True)
            tmpdir = tempfile.mkdtemp(dir=base)
        else:
            tmpdir = None
    else:
        tmpdir = None
    res = bass_utils.run_bass_kernel_spmd(
        nc, in_maps, core_ids=list(range(N_CORES)), trace=trace, tmpdir=tmpdir
    )
    LAST_RESULTS = res

    outT_full = np.concatenate(
        [np.asarray(res.results[c]["outT"]) for c in range(N_CORES)], axis=0
    )  # [D_OUT, M_TOT] bf16
    out = (
        np.ascontiguousarray(outT_full.T).astype(np.float32).reshape(orig_shape)
    )
    return out


# revision 7
# speedup vs baseline: 1.3455x; 1.0404x over previous
"""BitLinear kernel for Trainium2 (8 NeuronCores, tensor-parallel).

Computes: out = x @ (sign(w) * mean(|w|, axis=1, keepdims=True)).T
  x      : [4, 2048, 4096] f32
  weight : [4096, 4096] f32
  out    : [4, 2048, 4096] f32

Strategy: shard weight rows (out features) 8-way; each core computes a
[512, 8192] feature-major output shard.

Mixed-precision contraction (PE roofline is the bottleneck): the first
18 k-tiles run as bf16 matmuls (216ns per 128x512), the last 14 k-tiles
run as 7 fp8e4 DoubleRow pair-matmuls (2 k-tiles per instruction at
~241ns — 2x the contraction per instruction at +13% per-instruction
cost). sign(w) is exactly representable in both bf16 and fp8e4, so all
quantization error comes from the e4m3 x tiles: measured end-to-end
rel-err 1.76e-2 (gate 2e-2). Host computes signs and f32 scales; the
device applies the per-feature scale while evicting PSUM and stores the
output shard in bf16 (negligible extra error, half the store traffic).

Layouts (host-pretiled so every DMA is a contiguous 256KB chunk with
2KB-per-partition packets):
  xTp [8 pairs, 18, 128, 1024] bf16 — x.T k-tile chunks per 1024-token
      pair (two 512 blocks).
  xF  [8 pairs, 7, 128, 2048] fp8e4 — DoubleRow pairs: per partition
      [ktA blk0 512 | ktA blk1 512 | ktB blk0 | ktB blk1]; the matmul
      rhs AP is [128, 2 slots, 512] with slot stride 1024.
  sgB [128, 9216] bf16, sgF [128, 7168] fp8e4 — sign(w).T tiles,
      n-shard-major per k-tile; DR lhsT AP is [128, 2, 128], slot
      stride 512.
  sc  [128, 4] f32 — per-feature scales as n-tile columns.
  outT[512, 8192] bf16 — feature-major shard.

The matmul stream (16 blocks x 4 n-tiles x (18 bf16 + 7 DR)) is
~357us of PE time. x/sign loads chain in emission order on the sync
DMA queue (startup is HBM-paced); output stores ride the scalar
queue. Pair 0 is computed j-outer across both blocks (8 PSUM banks)
so the PE keeps pace with the HBM-limited startup stream.
"""

import os
from contextlib import ExitStack

import numpy as np
import ml_dtypes

import concourse.bass as bass
import concourse.mybir as mybir
import concourse.tile as tile
from concourse import bacc, bass_utils

P = 128                 # SBUF partitions / PE array dim
D_IN = 4096             # contraction dim (in features)
D_OUT = 4096            # out features
M_TOT = 8192            # tokens (4*2048)
N_CORES = 8
N_SHARD = D_OUT // N_CORES      # 512 out features per core
K_TILES = D_IN // P             # 32
NB = 16                         # bf16 k-tiles (0..NB-1)
NFP = (K_TILES - NB) // 2       # 8 fp8 DoubleRow k-tile pairs (NB..31)
M_BLK = 512                     # moving free dim per matmul
M_BLKS = M_TOT // M_BLK         # 16
M_PAIRS = M_BLKS // 2           # 8 (x is loaded in block pairs)
N_TILES = N_SHARD // P          # 4
PAIR_W = 2 * M_BLK              # 1024

_CACHE = {}
LAST_RESULTS = None  # BassKernelResults of the most recent run (for test harness)


def _install_ntff_hook():
    """Register the ctypes NTFF profiling hook under antenv.axon_hooks so
    run_bass_kernel_spmd(trace=True) can capture device profiles under axon.
    No-op if already present or the .so lacks the symbols."""
    import contextlib
    import ctypes
    import sys
    import types

    try:
        from antenv.axon_hooks import get_axon_ntff_profile_hook  # noqa: F401

        return True
    except ImportError:
        pass

    so_path = "/opt/axon/libaxon_pjrt.so"
    if not os.path.exists(so_path):
        return False
    lib = ctypes.CDLL(so_path)
    if not hasattr(lib, "axon_start_nrt_profile"):
        return False
    lib.axon_start_nrt_profile.argtypes = [
        ctypes.POINTER(ctypes.c_int64),
        ctypes.c_size_t,
    ]
    lib.axon_start_nrt_profile.restype = ctypes.c_int64
    lib.axon_stop_nrt_profile.argtypes = [ctypes.c_char_p]
    lib.axon_stop_nrt_profile.restype = ctypes.c_int64

    @contextlib.contextmanager
    def _hook(output_dir, device_ids):
        import jax

        jax.devices()
        if device_ids:
            ids = (ctypes.c_int64 * len(device_ids))(*device_ids)
            rc = lib.axon_start_nrt_profile(ids, len(device_ids))
        else:
            rc = lib.axon_start_nrt_profile(None, 0)
        if rc != 0:
            raise RuntimeError(f"axon_start_nrt_profile rc={rc}")
        try:
            yield
        finally:
            n = lib.axon_stop_nrt_profile(str(output_dir).encode())
            print(f"ntff profile: {n} file(s) written to {output_dir}")

    mod = types.ModuleType("antenv.axon_hooks")
    _state = {"hook": _hook}
    mod.set_axon_ntff_profile_hook = lambda h: _state.__setitem__("hook", h)
    mod.get_axon_ntff_profile_hook = lambda: _state["hook"]
    sys.modules["antenv.axon_hooks"] = mod
    import antenv

    antenv.axon_hooks = mod

    # artifact upload reaches for a cloud bucket that isn't available here
    bass_utils.upload_artifacts = lambda tmpdir: f"local:{tmpdir}"
    return True


def _build_nc():
    nc = bacc.Bacc(
        "TRN2", target_bir_lowering=False, debug=False, num_devices=N_CORES,
        enable_partition_id=False,
    )
    xTp = nc.dram_tensor(
        "xTp", [M_PAIRS, NB, P, PAIR_W], mybir.dt.bfloat16, kind="ExternalInput"
    )
    xF = nc.dram_tensor(
        "xF", [M_PAIRS, NFP, P, 2 * PAIR_W], mybir.dt.float8e4,
        kind="ExternalInput",
    )
    sgB = nc.dram_tensor(
        "sgB", [P, NB * N_SHARD], mybir.dt.bfloat16, kind="ExternalInput"
    )
    sgF = nc.dram_tensor(
        "sgF", [P, NFP * 2 * N_SHARD], mybir.dt.float8e4, kind="ExternalInput"
    )
    sc = nc.dram_tensor("sc", [P, N_TILES], mybir.dt.float32, kind="ExternalInput")
    outT = nc.dram_tensor(
        "outT", [N_SHARD, M_TOT], mybir.dt.bfloat16, kind="ExternalOutput"
    )

    with tile.TileContext(nc) as tc, ExitStack() as ctx:
        spool = ctx.enter_context(tc.tile_pool(name="scales", bufs=1))
        sgpool = ctx.enter_context(tc.tile_pool(name="sign", bufs=1))
        xpoolB = ctx.enter_context(tc.tile_pool(name="xb", bufs=2))
        xpoolF = ctx.enter_context(tc.tile_pool(name="xf", bufs=2))
        opool = ctx.enter_context(tc.tile_pool(name="oblk", bufs=8))
        ppool = ctx.enter_context(tc.tile_pool(name="psum", bufs=8, space="PSUM"))

        # sync queue = all input loads, chained in emission order so the
        # FIFO order is deterministic and startup is HBM-paced in the
        # order the PE consumes tiles. scalar queue = output stores.
        prev_sync_dma = [None]

        def sync_load(dst, src):
            dma = nc.sync.dma_start(dst, src)
            if prev_sync_dma[0] is not None:
                tile.add_dep_helper(
                    dma.ins, prev_sync_dma[0].ins, sync=False,
                    reason="sync DMA queue emission order",
                )
            prev_sync_dma[0] = dma
            return dma

        # Persistent sign tiles + scales
        sgB_t = sgpool.tile([P, NB, N_SHARD], mybir.dt.bfloat16)
        sgF_t = sgpool.tile([P, NFP, 2, N_SHARD], mybir.dt.float8e4)
        sct = spool.tile([P, N_TILES], mybir.dt.float32)

        def issue_x_pair(q, xb=None, xf=None):
            if xb is None:
                xb = xpoolB.tile([P, NB, PAIR_W], mybir.dt.bfloat16, tag="xb")
            if xf is None:
                xf = xpoolF.tile([P, NFP, 2, PAIR_W], mybir.dt.float8e4, tag="xf")
            for j in range(NB):
                sync_load(xb[:, j, :], xTp[q, j, :, :])
            for jj in range(NFP):
                sync_load(xf[:, jj, :, :], xF[q, jj, :, :])
            return xb, xf

        def mm_b(pss, xb, b, ni, j):
            nc.tensor.matmul(
                pss[ni][:],
                sgB_t[:, j, ni * P : (ni + 1) * P],
                xb[:, j, b * M_BLK : (b + 1) * M_BLK],
                start=(j == 0),
                stop=False,
            )

        def mm_f(pss, xf, b, ni, jj):
            nc.tensor.matmul(
                pss[ni][:],
                sgF_t[:, jj, :, ni * P : (ni + 1) * P],
                xf[:, jj, :, b * M_BLK : (b + 1) * M_BLK],
                start=False,
                stop=(jj == NFP - 1),
                perf_mode=mybir.MatmulPerfMode.DoubleRow,
            )

        def evict_block(pss, opair, b, last=False):
            # Evictions alternate between the scalar and vector engines so
            # the per-block eviction chain (and the kernel tail) is half as
            # long. Stores (issued on b==1 once the [128,1024] pair tile is
            # complete) ride the scalar queue; the final pair splits store
            # triggers across the sync queue (drained of x loads by then).
            for ni in range(N_TILES):
                dst = opair[ni][:, b * M_BLK : (b + 1) * M_BLK]
                if ni % 2 == 0:
                    nc.scalar.mul(dst, pss[ni][:], sct[:, ni : ni + 1])
                else:
                    nc.vector.tensor_scalar_mul(dst, pss[ni][:], sct[:, ni : ni + 1])

        def store_pair(q, opair):
            for ni in range(N_TILES):
                dst = outT[ni * P : (ni + 1) * P, q * PAIR_W : (q + 1) * PAIR_W]
                nc.scalar.dma_start(dst, opair[ni][:])

        def store_half(q, opair, b, ni, queue):
            dst = outT[
                ni * P : (ni + 1) * P,
                q * PAIR_W + b * M_BLK : q * PAIR_W + (b + 1) * M_BLK,
            ]
            queue.dma_start(dst, opair[ni][:, b * M_BLK : (b + 1) * M_BLK])

        # ---- HAM warmup: dummy matmuls with no DMA dependencies run during
        # the DMA-bound prologue (~7-11us) and un-throttle the PE clock gate
        # (cold 1.2GHz -> warm 2.4GHz needs ~3.4us of sustained PE activity)
        # before the first real matmul's operands land.
        warm = spool.tile([P, 5 * P], mybir.dt.bfloat16)
        nc.gpsimd.memset(warm[:], 0.0)
        ps_warm = ppool.tile([P, M_BLK], mybir.dt.float32, tag="ps", name="ps_warm")
        for _ in range(9):
            nc.tensor.matmul(
                ps_warm[:], warm[:, 0:P], warm[:, P : P + M_BLK],
                start=True, stop=True,
            )

        # ---- Prologue: chain sign chunks just ahead of the x tiles that
        # consume them so the earliest matmuls are fed with minimal latency.
        xb0 = xpoolB.tile([P, NB, PAIR_W], mybir.dt.bfloat16, tag="xb")
        xf0 = xpoolF.tile([P, NFP, 2, PAIR_W], mybir.dt.float8e4, tag="xf")
        # k-tile 0 signs ride the (idle) scalar queue in parallel with the
        # first x halves on the sync queue, so matmul #0's two operands
        # land concurrently instead of serially.
        nc.scalar.dma_start(sgB_t[:, 0, :], sgB[:, 0:N_SHARD])
        sync_load(xb0[:, 0, 0:M_BLK], xTp[0, 0, :, 0:M_BLK])
        sync_load(xb0[:, 0, M_BLK:PAIR_W], xTp[0, 0, :, M_BLK:])
        # remaining bf16 signs in 2-k-tile chunks, interleaved with pair-0 x
        j = 1
        while j < NB:
            j2 = min(j + 2, NB)
            sync_load(sgB_t[:, j:j2, :], sgB[:, j * N_SHARD : j2 * N_SHARD])
            for jx in range(j, j2):
                sync_load(xb0[:, jx, :], xTp[0, jx, :, :])
            j = j2
        # fp8 signs in 2-pair chunks interleaved with pair-0 fp8 x
        jj = 0
        while jj < NFP:
            jj2 = min(jj + 2, NFP)
            sync_load(
                sgF_t[:, jj:jj2, :, :],
                sgF[:, jj * 2 * N_SHARD : jj2 * 2 * N_SHARD],
            )
            for jx in range(jj, jj2):
                sync_load(xf0[:, jx, :, :], xF[0, jx, :, :])
            jj = jj2
        # scales ride behind pair-0 (only needed at first eviction)
        sync_load(sct[:], sc[:, :])

        # ---- Main loop
        for q in range(M_PAIRS):
            xb, xf = (xb0, xf0) if q == 0 else issue_x_pair(q)
            opair = [
                opool.tile([P, PAIR_W], mybir.dt.bfloat16, tag="op", name=f"op{ni}")
                for ni in range(N_TILES)
            ]
            if q == 0:
                # j-outer across BOTH blocks (8 PSUM banks) so the PE keeps
                # pace with the HBM-limited startup stream.
                pss2 = [
                    [
                        ppool.tile(
                            [P, M_BLK], mybir.dt.float32, tag="ps",
                            name=f"ps_{b}_{ni}",
                        )
                        for ni in range(N_TILES)
                    ]
                    for b in range(2)
                ]
                for j in range(NB):
                    for b in range(2):
                        for ni in range(N_TILES):
                            mm_b(pss2[b], xb, b, ni, j)
                for jj in range(NFP):
                    for b in range(2):
                        for ni in range(N_TILES):
                            mm_f(pss2[b], xf, b, ni, jj)
                for b in range(2):
                    evict_block(pss2[b], opair, b)
                store_pair(q, opair)
            elif q < M_PAIRS - 1:
                for b in range(2):
                    pss = [
                        ppool.tile(
                            [P, M_BLK], mybir.dt.float32, tag="ps", name=f"ps{ni}"
                        )
                        for ni in range(N_TILES)
                    ]
                    for j in range(NB):
                        for ni in range(N_TILES):
                            mm_b(pss, xb, b, ni, j)
                    for jj in range(NFP):
                        for ni in range(N_TILES):
                            mm_f(pss, xf, b, ni, jj)
                    evict_block(pss, opair, b)
                store_pair(q, opair)
            else:
                # Final pair: store each 512-block as soon as its eviction
                # completes (instead of waiting for the full 1024 pair tile)
                # and spread the tail stores across four DMA queues so the
                # kernel end isn't serialized on one ring.
                tailq = [nc.scalar, nc.gpsimd, nc.sync, nc.scalar]
                for b in range(2):
                    pss = [
                        ppool.tile(
                            [P, M_BLK], mybir.dt.float32, tag="ps", name=f"ps{ni}"
                        )
                        for ni in range(N_TILES)
                    ]
                    if b == 0:
                        for j in range(NB):
                            for ni in range(N_TILES):
                                mm_b(pss, xb, b, ni, j)
                        for jj in range(NFP):
                            for ni in range(N_TILES):
                                mm_f(pss, xf, b, ni, jj)
                        evict_block(pss, opair, b)
                        for ni in range(N_TILES):
                            store_half(q, opair, b, ni, tailq[ni])
                    else:
                        # ni-outer: each n-tile's stop matmul lands early, so
                        # its eviction + store overlap the remaining matmuls
                        # instead of serializing after the last one.
                        for ni in range(N_TILES):
                            for j in range(NB):
                                mm_b(pss, xb, b, ni, j)
                            for jj in range(NFP):
                                mm_f(pss, xf, b, ni, jj)
                        for ni in range(N_TILES):
                            dst = opair[ni][:, M_BLK:PAIR_W]
                            if ni % 2 == 0:
                                nc.scalar.mul(dst, pss[ni][:], sct[:, ni : ni + 1])
                            else:
                                nc.vector.tensor_scalar_mul(
                                    dst, pss[ni][:], sct[:, ni : ni + 1]
                                )
                            store_half(q, opair, b, ni, tailq[ni])

    nc.compile()
    return nc


def kernel(x, weight):
    global LAST_RESULTS
    nc = _CACHE.get("nc")
    if nc is None:
        nc = _CACHE["nc"] = _build_nc()

    x = np.asarray(x)
    weight = np.asarray(weight)
    orig_shape = x.shape

    KB = NB * P  # 2304 contraction cols in bf16

    # Host-side layout: x.T pre-tiled; bf16 for k-tiles 0..NB-1, e4m3 for
    # the DoubleRow k-tile pairs. Each (pair, chunk) DMA source is a fully
    # contiguous 256KB read with 2KB-per-partition packets.
    xT = x.reshape(M_TOT, D_IN).T  # [D_IN, M_TOT] view
    xTp = np.ascontiguousarray(
        xT[:KB].reshape(NB, P, M_PAIRS, PAIR_W)
        .transpose(2, 0, 1, 3)
        .astype(ml_dtypes.bfloat16)
    )  # [M_PAIRS, NB, P, 1024]
    # [NFP, 2, P, M_PAIRS, 1024] -> [M_PAIRS, NFP, P, 2, 1024] -> flat 2048
    xF = np.ascontiguousarray(
        xT[KB:].reshape(NFP, 2, P, M_PAIRS, PAIR_W)
        .transpose(3, 0, 2, 1, 4)
        .reshape(M_PAIRS, NFP, P, 2 * PAIR_W)
        .astype(ml_dtypes.float8_e4m3fn)
    )

    SgT = np.sign(weight.T)  # [D_IN, D_OUT] f32, sign exact
    s_full = np.abs(weight.astype(np.float64)).mean(axis=1).astype(np.float32)

    in_maps = []
    for c in range(N_CORES):
        n0 = c * N_SHARD
        shard = SgT[:, n0 : n0 + N_SHARD]  # [D_IN, 512]
        # sgB[p, j*512+n] = sign(wT[j*128+p, n0+n])
        sgB = np.ascontiguousarray(
            shard[:KB].reshape(NB, P, N_SHARD)
            .transpose(1, 0, 2)
            .reshape(P, NB * N_SHARD)
            .astype(ml_dtypes.bfloat16)
        )
        # sgF[p, jj*1024 + t*512 + n] = sign(wT[(NB+2jj+t)*128+p, n0+n])
        sgF = np.ascontiguousarray(
            shard[KB:].reshape(NFP, 2, P, N_SHARD)
            .transpose(2, 0, 1, 3)
            .reshape(P, NFP * 2 * N_SHARD)
            .astype(ml_dtypes.float8_e4m3fn)
        )
        scl = np.ascontiguousarray(
            s_full[n0 : n0 + N_SHARD].reshape(N_TILES, P).T
        )  # [128, 4] f32
        in_maps.append({"xTp": xTp, "xF": xF, "sgB": sgB, "sgF": sgF, "sc": scl})

    trace = bool(int(os.environ.get("BITLIN_TRACE", "0")))
    if trace:
        trace = _install_ntff_hook()
        base = os.environ.get("BITLIN_TRACE_DIR") or None
        if base:
            import tempfile

            os.makedirs(base, exist_ok=True)
            tmpdir = tempfile.mkdtemp(dir=base)
        else:
            tmpdir = None
    else:
        tmpdir = None
    res = bass_utils.run_bass_kernel_spmd(
        nc, in_maps, core_ids=list(range(N_CORES)), trace=trace, tmpdir=tmpdir
    )
    LAST_RESULTS = res

    outT_full = np.concatenate(
        [np.asarray(res.results[c]["outT"]) for c in range(N_CORES)], axis=0
    )  # [D_OUT, M_TOT] bf16
    out = (
        np.ascontiguousarray(outT_full.T).astype(np.float32).reshape(orig_shape)
    )
    return out
